# revision 1
# baseline (speedup 1.0000x reference)
"""STFT kernel for Trainium2 (8 NeuronCores, batch-parallel).

Computes the equivalent of:
    xp = reflect_pad(x, 512)
    frames[b, f, n] = xp[b, 256*f + n] * window[n]      (f < 1025, n < 1024)
    spec = rfft(frames, axis=-1)                        -> [B, 1025, 513]
    out  = transpose(spec, (0, 2, 1))                   -> [B, 513, 1025] c64

Implementation: the windowed DFT is two fp32 TensorE matmuls per frequency
tile (cos and -sin stationary matrices with the window folded in), with the
contraction over the 1024 frame samples.  The trick that avoids materializing
overlapping frames: with hop 256, frames^T[(128*t + p), f] for K-tile t is
the transposed hop-matrix Y^T[r, g] = xp[256*g + r] shifted by t//2 in the
free (frame) axis.  So each batch needs only the 256 x 1028 transposed view
of xp in SBUF, and all 8 K-tiles of the matmul read it at different offsets.

Batch dim (16) is sharded across the 8 cores, 2 batches each; no
cross-device communication.
"""

from contextlib import ExitStack

import numpy as np

import concourse.mybir as mybir
import concourse.tile as tile
from concourse import bacc
from concourse.bass_utils import run_bass_kernel_spmd

NFFT, HOP, PAD = 1024, 256, 512
B, T = 16, 262144
NCORES = 8
BC = B // NCORES                 # batches per core
G = (T + 2 * PAD) // HOP         # 1028 hop blocks per padded row
NF = (T + 2 * PAD - NFFT) // HOP + 1   # 1025 frames
KF = NFFT // 2 + 1               # 513 one-sided freqs
KTILES = 5                       # ceil(513/128); last tile has 1 valid row
KPAD = KTILES * 128              # 640
CHUNKS = [(0, 342), (342, 342), (684, 341)]   # frame chunks (sum = 1025)

_cache = {}


def _build():
    nc = bacc.Bacc(
        "TRN2", target_bir_lowering=False, debug=False, num_devices=NCORES
    )
    f32 = mybir.dt.float32
    xt_d = nc.dram_tensor("xt", [BC, 256, G], f32, kind="ExternalInput")
    wc_d = nc.dram_tensor("wc", [128, 8, KPAD], f32, kind="ExternalInput")
    ws_d = nc.dram_tensor("ws", [128, 8, KPAD], f32, kind="ExternalInput")
    out_d = nc.dram_tensor("out", [BC, KF, 2 * NF], f32, kind="ExternalOutput")

    with tile.TileContext(nc) as tc, ExitStack() as ctx:
        consts = ctx.enter_context(tc.tile_pool(name="consts", bufs=1))
        xpool = ctx.enter_context(tc.tile_pool(name="x", bufs=2 * BC))
        opool = ctx.enter_context(tc.tile_pool(name="o", bufs=4))
        ppool = ctx.enter_context(tc.tile_pool(name="psum", bufs=4, space="PSUM"))

        wc_sb = consts.tile([128, 8, KPAD], f32)
        nc.sync.dma_start(wc_sb[:], wc_d.ap())
        ws_sb = consts.tile([128, 8, KPAD], f32)
        nc.sync.dma_start(ws_sb[:], ws_d.ap())

        for b in range(BC):
            xs = []
            for h in range(2):
                xh = xpool.tile([128, G], f32, name=f"x{h}")
                nc.sync.dma_start(xh[:], xt_d.ap()[b, 128 * h : 128 * (h + 1), :])
                xs.append(xh)
            for m in range(KTILES):
                mc = slice(128 * m, 128 * (m + 1))
                for f0, fn in CHUNKS:
                    ps_re = ppool.tile([128, 512], f32, name="psre")[:, :fn]
                    for t in range(8):
                        nc.tensor.matmul(
                            ps_re,
                            wc_sb[:, t, mc],
                            xs[t % 2][:, t // 2 + f0 : t // 2 + f0 + fn],
                            start=(t == 0),
                            stop=(t == 7),
                        )
                    if m < 4:
                        ps_im = ppool.tile([128, 512], f32, name="psim")[:, :fn]
                        for t in range(8):
                            nc.tensor.matmul(
                                ps_im,
                                ws_sb[:, t, mc],
                                xs[t % 2][:, t // 2 + f0 : t // 2 + f0 + fn],
                                start=(t == 0),
                                stop=(t == 7),
                            )
                        ot = opool.tile([128, 2 * fn], f32, name="ot")
                        nc.vector.tensor_copy(ot[:, 0::2], ps_re)
                        nc.scalar.copy(ot[:, 1::2], ps_im)
                        nc.sync.dma_start(
                            out_d.ap()[b, mc, 2 * f0 : 2 * (f0 + fn)], ot[:]
                        )
                    else:
                        # Nyquist row (k=512): imag == 0 exactly (sin(pi*n)=0),
                        # only partition 0 of the tile is a real frequency.
                        ot = opool.tile([1, 2 * fn], f32, name="otn")
                        nc.vector.tensor_copy(ot[:1, 0::2], ps_re[:1, :])
                        nc.scalar.mul(ot[:1, 1::2], ps_re[:1, :], 0.0)
                        nc.sync.dma_start(
                            out_d.ap()[b, 512:513, 2 * f0 : 2 * (f0 + fn)], ot[:1, :]
                        )
    nc.compile()
    return nc


def _weights(window):
    w = np.asarray(window, np.float64)
    n = np.arange(NFFT, dtype=np.float64)
    k = np.arange(KPAD, dtype=np.float64)
    ang = 2.0 * np.pi * np.outer(n, k) / NFFT
    wc = w[:, None] * np.cos(ang)
    ws = -w[:, None] * np.sin(ang)
    wc[:, KF:] = 0.0
    ws[:, KF:] = 0.0
    wc = np.ascontiguousarray(
        wc.reshape(8, 128, KPAD).transpose(1, 0, 2), dtype=np.float32
    )
    ws = np.ascontiguousarray(
        ws.reshape(8, 128, KPAD).transpose(1, 0, 2), dtype=np.float32
    )
    return wc, ws


def prep_inputs(x, window):
    """Host-side shard/layout prep: per-core input maps."""
    xp = np.pad(np.asarray(x, np.float32), ((0, 0), (PAD, PAD)), mode="reflect")
    xt = np.ascontiguousarray(xp.reshape(B, G, HOP).transpose(0, 2, 1))
    wc, ws = _weights(window)
    return [
        {"xt": xt[i * BC : (i + 1) * BC], "wc": wc, "ws": ws}
        for i in range(NCORES)
    ]


def get_nc():
    nc = _cache.get("nc")
    if nc is None:
        nc = _build()
        _cache["nc"] = nc
    return nc


def kernel(x, window, _trace=False, _trace_kwargs=None):
    nc = get_nc()
    in_maps = prep_inputs(x, window)
    res = run_bass_kernel_spmd(
        nc, in_maps, list(range(NCORES)), trace=_trace, **(_trace_kwargs or {})
    )
    _cache["last_results"] = res
    out = np.concatenate([r["out"] for r in res.results], axis=0)
    return np.ascontiguousarray(out).view(np.complex64)


# revision 7
# speedup vs baseline: 3.1440x; 3.1440x over previous
"""STFT kernel for Trainium2 (8 NeuronCores, batch-parallel).

Computes the equivalent of:
    xp = reflect_pad(x, 512)
    frames[b, f, n] = xp[b, 256*f + n] * window[n]      (f < 1025, n < 1024)
    spec = rfft(frames, axis=-1)                        -> [B, 1025, 513]
    out  = transpose(spec, (0, 2, 1))                   -> [B, 513, 1025] c64

Implementation: the windowed DFT is two TensorE matmuls per frequency tile
(cos and -sin stationary matrices with the window folded in), contraction
over the 1024 frame samples.  The trick that avoids materializing the 4x
overlapping frames: with hop 256, frames^T[(128*t + p), f] for K-tile t is
the transposed hop-matrix Y^T[r, g] = xp[256*g + r] shifted by t//2 in the
free (frame) axis.  So each batch needs only the 256 x 1028 transposed view
of xp in SBUF, and all 8 K-tiles of the matmul read it at different offsets.

Matmuls run in float32r (single-pass reduced-precision fp32) — full fp32 is
a 2-pass HI/LO operation on TRN2, 2x the TensorE time.

Batch dim (16) is sharded across the 8 cores, 2 batches each; no
cross-device communication.
"""

from contextlib import ExitStack

import numpy as np

import concourse.mybir as mybir
import concourse.tile as tile
from concourse import bacc
from concourse.bass_utils import run_bass_kernel_spmd

NFFT, HOP, PAD = 1024, 256, 512
B, T = 16, 262144
NCORES = 8
BC = B // NCORES                 # batches per core
G = (T + 2 * PAD) // HOP         # 1028 hop blocks per padded row
GP = G + 2                       # padded to 1030 so the garbage tail frame is in-bounds
NF = (T + 2 * PAD - NFFT) // HOP + 1   # 1025 frames
KF = NFFT // 2 + 1               # 513 one-sided freqs
KTILES = 5                       # ceil(513/128); last tile has 1 valid row
# (f0, fn, valid): matmul frame-chunks; fn is even (fp32r requires even
# innermost counts), the last chunk computes one garbage frame (f=1025)
# that is simply not stored.  valid frames sum to 1025.
CHUNKS = [(0, 344, 344), (344, 342, 342), (686, 340, 339)]

_cache = {}


def _build():
    nc = bacc.Bacc(
        "TRN2", target_bir_lowering=False, debug=False, num_devices=NCORES
    )
    f32 = mybir.dt.float32
    f32r = mybir.dt.float32r
    xt_d = nc.dram_tensor("xt", [BC, 256, GP], f32r, kind="ExternalInput")
    wc_d = nc.dram_tensor("wc", [KTILES, 128, 8, 128], f32r, kind="ExternalInput")
    ws_d = nc.dram_tensor("ws", [KTILES, 128, 8, 128], f32r, kind="ExternalInput")
    out_d = nc.dram_tensor("out", [BC, KF, 2 * NF], f32, kind="ExternalOutput")

    with tile.TileContext(nc) as tc, ExitStack() as ctx:
        consts = ctx.enter_context(tc.tile_pool(name="consts", bufs=1))
        xpool = ctx.enter_context(tc.tile_pool(name="x", bufs=2 * BC))
        opool = ctx.enter_context(tc.tile_pool(name="o", bufs=4))
        ppool = ctx.enter_context(tc.tile_pool(name="psum", bufs=4, space="PSUM"))

        # batch-0 input first so the first matmul chain can start early,
        # then the m=0 weights, then everything else (prefetch).
        xs = {}
        for b in range(BC):
            for h in range(2):
                xh = xpool.tile([128, GP], f32r, name=f"x{b}{h}")
                xs[(b, h)] = xh
        for h in range(2):
            nc.sync.dma_start(xs[(0, h)][:], xt_d.ap()[0, 128 * h : 128 * (h + 1), :])

        wcs, wss = [], []
        for m in range(KTILES):
            wcs.append(consts.tile([128, 8, 128], f32r, name=f"wc{m}"))
            wss.append(consts.tile([128, 8, 128], f32r, name=f"ws{m}"))
        nc.sync.dma_start(wcs[0][:], wc_d.ap()[0])
        nc.sync.dma_start(wss[0][:], ws_d.ap()[0])
        for b in range(1, BC):
            for h in range(2):
                nc.sync.dma_start(
                    xs[(b, h)][:], xt_d.ap()[b, 128 * h : 128 * (h + 1), :]
                )
        for m in range(1, KTILES):
            nc.sync.dma_start(wcs[m][:], wc_d.ap()[m])
            nc.sync.dma_start(wss[m][:], ws_d.ap()[m])

        for b in range(BC):
            for m in range(KTILES):
                for f0, fn, valid in CHUNKS:
                    ps_re = ppool.tile([128, 512], f32, name="psre")[:, :fn]
                    for t in range(8):
                        nc.tensor.matmul(
                            ps_re,
                            wcs[m][:, t, :],
                            xs[(b, t % 2)][:, t // 2 + f0 : t // 2 + f0 + fn],
                            start=(t == 0),
                            stop=(t == 7),
                        )
                    if m < 4:
                        ps_im = ppool.tile([128, 512], f32, name="psim")[:, :fn]
                        for t in range(8):
                            nc.tensor.matmul(
                                ps_im,
                                wss[m][:, t, :],
                                xs[(b, t % 2)][:, t // 2 + f0 : t // 2 + f0 + fn],
                                start=(t == 0),
                                stop=(t == 7),
                            )
                        ot = opool.tile([128, 2 * fn], f32, name="ot")
                        nc.vector.tensor_copy(ot[:, 0::2], ps_re)
                        nc.scalar.copy(ot[:, 1::2], ps_im)
                        nc.sync.dma_start(
                            out_d.ap()[b, 128 * m : 128 * (m + 1), 2 * f0 : 2 * (f0 + valid)],
                            ot[:, : 2 * valid],
                        )
                    else:
                        # Nyquist row (k=512): imag == 0 exactly (sin(pi*n)=0),
                        # only partition 0 of this tile is a real frequency.
                        ot = opool.tile([1, 2 * fn], f32, name="otn")
                        nc.vector.tensor_copy(ot[:1, 0::2], ps_re[:1, :])
                        nc.scalar.mul(ot[:1, 1::2], ps_re[:1, :], 0.0)
                        nc.sync.dma_start(
                            out_d.ap()[b, 512:513, 2 * f0 : 2 * (f0 + valid)],
                            ot[:1, : 2 * valid],
                        )
    nc.compile()
    return nc


def _weights(window):
    w = np.asarray(window, np.float64)
    n = np.arange(NFFT, dtype=np.float64)
    k = np.arange(KTILES * 128, dtype=np.float64)
    ang = 2.0 * np.pi * np.outer(n, k) / NFFT
    wc = w[:, None] * np.cos(ang)
    ws = -w[:, None] * np.sin(ang)
    wc[:, KF:] = 0.0
    ws[:, KF:] = 0.0
    # [n, k] -> [m, p, t, kk] with n = 128*t + p, k = 128*m + kk
    wc = np.ascontiguousarray(
        wc.reshape(8, 128, KTILES, 128).transpose(2, 1, 0, 3), dtype=np.float32
    )
    ws = np.ascontiguousarray(
        ws.reshape(8, 128, KTILES, 128).transpose(2, 1, 0, 3), dtype=np.float32
    )
    return wc, ws


def prep_inputs(x, window):
    """Host-side shard/layout prep: per-core input maps."""
    xp = np.pad(np.asarray(x, np.float32), ((0, 0), (PAD, PAD)), mode="reflect")
    xt = np.zeros((B, HOP, GP), np.float32)
    xt[:, :, :G] = xp.reshape(B, G, HOP).transpose(0, 2, 1)
    wc, ws = _weights(window)
    return [
        {"xt": xt[i * BC : (i + 1) * BC], "wc": wc, "ws": ws}
        for i in range(NCORES)
    ]


def get_nc():
    nc = _cache.get("nc")
    if nc is None:
        nc = _build()
        _cache["nc"] = nc
    return nc


def kernel(x, window, _trace=False, _trace_kwargs=None):
    nc = get_nc()
    in_maps = prep_inputs(x, window)
    res = run_bass_kernel_spmd(
        nc, in_maps, list(range(NCORES)), trace=_trace, **(_trace_kwargs or {})
    )
    _cache["last_results"] = res
    out = np.concatenate([r["out"] for r in res.results], axis=0)
    return np.ascontiguousarray(out).view(np.complex64)


# revision 8
# speedup vs baseline: 3.4242x; 1.0891x over previous
"""STFT kernel for Trainium2 (8 NeuronCores, batch-parallel).

Computes the equivalent of:
    xp = reflect_pad(x, 512)
    frames[b, f, n] = xp[b, 256*f + n] * window[n]      (f < 1025, n < 1024)
    spec = rfft(frames, axis=-1)                        -> [B, 1025, 513]
    out  = transpose(spec, (0, 2, 1))                   -> [B, 513, 1025] c64

Implementation: the windowed DFT is two TensorE matmuls per frequency tile
(cos and -sin stationary matrices with the window folded in), contraction
over the 1024 frame samples.  The trick that avoids materializing the 4x
overlapping frames: with hop 256, frames^T[(128*t + p), f] for K-tile t is
the transposed hop-matrix Y^T[r, g] = xp[256*g + r] shifted by t//2 in the
free (frame) axis.  So each batch needs only the 256 x 1028 transposed view
of xp in SBUF, and all 8 K-tiles of the matmul read it at different offsets.

Matmul operands are fp16 (fp32 accumulation in PSUM): full fp32 is a 2-pass
HI/LO operation on TRN2 (2x TensorE time) and fp32 weight loads cannot use
fast-weight-load.

Batch dim (16) is sharded across the 8 cores, 2 batches each; no
cross-device communication.
"""

from contextlib import ExitStack

import numpy as np

import concourse.mybir as mybir
import concourse.tile as tile
from concourse import bacc
from concourse.bass_utils import run_bass_kernel_spmd

NFFT, HOP, PAD = 1024, 256, 512
B, T = 16, 262144
NCORES = 8
BC = B // NCORES                 # batches per core
G = (T + 2 * PAD) // HOP         # 1028 hop blocks per padded row
GP = G + 2                       # padded to 1030 so the garbage tail frame is in-bounds
NF = (T + 2 * PAD - NFFT) // HOP + 1   # 1025 frames
KF = NFFT // 2 + 1               # 513 one-sided freqs
KTILES = 5                       # ceil(513/128); last tile has 1 valid row
# (f0, fn, valid): matmul frame-chunks; fn is even (fp32r requires even
# innermost counts), the last chunk computes one garbage frame (f=1025)
# that is simply not stored.  valid frames sum to 1025.
CHUNKS = [(0, 344, 344), (344, 342, 342), (686, 340, 339)]

_cache = {}


def _build():
    nc = bacc.Bacc(
        "TRN2", target_bir_lowering=False, debug=False, num_devices=NCORES
    )
    f32 = mybir.dt.float32
    f16 = mybir.dt.float16
    xt_d = nc.dram_tensor("xt", [BC, 256, GP], f16, kind="ExternalInput")
    wc_d = nc.dram_tensor("wc", [KTILES, 128, 8, 128], f16, kind="ExternalInput")
    ws_d = nc.dram_tensor("ws", [KTILES, 128, 8, 128], f16, kind="ExternalInput")
    out_d = nc.dram_tensor("out", [BC, KF, 2 * NF], f32, kind="ExternalOutput")

    with tile.TileContext(nc) as tc, ExitStack() as ctx:
        consts = ctx.enter_context(tc.tile_pool(name="consts", bufs=1))
        xpool = ctx.enter_context(tc.tile_pool(name="x", bufs=2 * BC))
        opool = ctx.enter_context(tc.tile_pool(name="o", bufs=4))
        ppool = ctx.enter_context(tc.tile_pool(name="psum", bufs=4, space="PSUM"))

        # batch-0 input first so the first matmul chain can start early,
        # then the m=0 weights, then everything else (prefetch).
        xs = {}
        for b in range(BC):
            for h in range(2):
                xh = xpool.tile([128, GP], f16, name=f"x{b}{h}")
                xs[(b, h)] = xh
        for h in range(2):
            nc.sync.dma_start(xs[(0, h)][:], xt_d.ap()[0, 128 * h : 128 * (h + 1), :])

        wcs, wss = [], []
        for m in range(KTILES):
            wcs.append(consts.tile([128, 8, 128], f16, name=f"wc{m}"))
            wss.append(consts.tile([128, 8, 128], f16, name=f"ws{m}"))
        for m in range(KTILES):
            nc.sync.dma_start(wcs[m][:], wc_d.ap()[m])
            nc.sync.dma_start(wss[m][:], ws_d.ap()[m])
        for b in range(1, BC):
            for h in range(2):
                nc.sync.dma_start(
                    xs[(b, h)][:], xt_d.ap()[b, 128 * h : 128 * (h + 1), :]
                )

        for b in range(BC):
            for m in range(KTILES):
                for f0, fn, valid in CHUNKS:
                    ps_re = ppool.tile([128, 512], f32, name="psre")[:, :fn]
                    for t in range(8):
                        nc.tensor.matmul(
                            ps_re,
                            wcs[m][:, t, :],
                            xs[(b, t % 2)][:, t // 2 + f0 : t // 2 + f0 + fn],
                            start=(t == 0),
                            stop=(t == 7),
                        )
                    if m < 4:
                        ps_im = ppool.tile([128, 512], f32, name="psim")[:, :fn]
                        for t in range(8):
                            nc.tensor.matmul(
                                ps_im,
                                wss[m][:, t, :],
                                xs[(b, t % 2)][:, t // 2 + f0 : t // 2 + f0 + fn],
                                start=(t == 0),
                                stop=(t == 7),
                            )
                        ot = opool.tile([128, 2 * fn], f32, name="ot")
                        nc.vector.tensor_copy(ot[:, 0::2], ps_re)
                        nc.scalar.copy(ot[:, 1::2], ps_im)
                        nc.sync.dma_start(
                            out_d.ap()[b, 128 * m : 128 * (m + 1), 2 * f0 : 2 * (f0 + valid)],
                            ot[:, : 2 * valid],
                        )
                    else:
                        # Nyquist row (k=512): imag == 0 exactly (sin(pi*n)=0),
                        # only partition 0 of this tile is a real frequency.
                        ot = opool.tile([1, 2 * fn], f32, name="otn")
                        nc.vector.tensor_copy(ot[:1, 0::2], ps_re[:1, :])
                        nc.scalar.mul(ot[:1, 1::2], ps_re[:1, :], 0.0)
                        nc.sync.dma_start(
                            out_d.ap()[b, 512:513, 2 * f0 : 2 * (f0 + valid)],
                            ot[:1, : 2 * valid],
                        )
    nc.compile()
    return nc


def _weights(window):
    w = np.asarray(window, np.float64)
    n = np.arange(NFFT, dtype=np.float64)
    k = np.arange(KTILES * 128, dtype=np.float64)
    ang = 2.0 * np.pi * np.outer(n, k) / NFFT
    wc = w[:, None] * np.cos(ang)
    ws = -w[:, None] * np.sin(ang)
    wc[:, KF:] = 0.0
    ws[:, KF:] = 0.0
    # [n, k] -> [m, p, t, kk] with n = 128*t + p, k = 128*m + kk
    wc = np.ascontiguousarray(
        wc.reshape(8, 128, KTILES, 128).transpose(2, 1, 0, 3), dtype=np.float16
    )
    ws = np.ascontiguousarray(
        ws.reshape(8, 128, KTILES, 128).transpose(2, 1, 0, 3), dtype=np.float16
    )
    return wc, ws


def prep_inputs(x, window):
    """Host-side shard/layout prep: per-core input maps."""
    xp = np.pad(np.asarray(x, np.float32), ((0, 0), (PAD, PAD)), mode="reflect")
    xt = np.zeros((B, HOP, GP), np.float16)
    xt[:, :, :G] = xp.reshape(B, G, HOP).transpose(0, 2, 1)
    wc, ws = _weights(window)
    return [
        {"xt": xt[i * BC : (i + 1) * BC], "wc": wc, "ws": ws}
        for i in range(NCORES)
    ]


def get_nc():
    nc = _cache.get("nc")
    if nc is None:
        nc = _build()
        _cache["nc"] = nc
    return nc


def kernel(x, window, _trace=False, _trace_kwargs=None):
    nc = get_nc()
    in_maps = prep_inputs(x, window)
    res = run_bass_kernel_spmd(
        nc, in_maps, list(range(NCORES)), trace=_trace, **(_trace_kwargs or {})
    )
    _cache["last_results"] = res
    out = np.concatenate([r["out"] for r in res.results], axis=0)
    return np.ascontiguousarray(out).view(np.complex64)
